# revision 18
# baseline (speedup 1.0000x reference)
"""Trainium2 Bass kernel for nn_MiningGNN (2-layer GAT message passing).

Sharding: nodes range-sharded across 8 cores; edges sharded by destination
owner (edge-parallel by dst range), grouped into 32-node dst buckets padded
to a uniform tile capacity so one SPMD program serves all cores.

Per conv layer each core computes its shard of the node table y1 = x1 @ R
(R an orthogonal Householder involution whose column 31 is att_src/|att_src|,
so y1[:,31] * |att_src| == a_src), all-gathers it, then streams its edges.
The per-edge gather uses ONE dma_gather per 4-bucket chunk (int16 indices
address 256-byte blocks of 4 consecutive nodes; src>>2 < 32768 fits int16),
instead of one indirect DMA per 128-edge tile — this removes the ~1us/tile
SWDGE fixed cost that dominated the per-tile version. The unneeded 3 nodes
per gathered block are masked by folding a per-quarter mask into ex: the
scatter one-hot matmul accumulates a 128-wide (4 quarters x 32 features)
numerator that is quarter-folded to 32 right out of PSUM. Aggregation stays
in the rotated basis; the un-rotation (h = hy @ R) is applied f-major fused
with bias+relu for conv1 and folded into the decoder weights for conv2.
segment_max is skipped (softmax is invariant; scores are tiny and clamped).
"""
import numpy as np

P = 128          # partitions / edge-tile height
B = 32           # dst-bucket width (nodes)
CG = 4           # buckets per edge-pipeline chunk
NCA = 448        # f-major node chunk (divides nloc)
NEG = 0.2        # leaky_relu slope
ZCLAMP = 30.0


# ----------------------------------------------------------------- host layout
def _build_layout(src, dst, attr, n_nodes, n_cores):
    """Sort edges by dst, shard by dst range, bucket and pad to uniform tiles."""
    nloc_raw = -(-n_nodes // n_cores)
    nbkt = -(-nloc_raw // B)
    nbkt = -(-nbkt // CG) * CG                         # multiple of CG
    while (nbkt * B) % NCA:
        nbkt += CG
    nloc = nbkt * B
    n_pad = nloc * n_cores

    order = np.argsort(dst, kind="stable")
    s_s = src[order]
    d_s = dst[order]
    a_s = attr[order]
    core_of = d_s // nloc
    gbkt = d_s // B
    bkt_loc = gbkt - core_of * nbkt
    counts = np.bincount(core_of * nbkt + bkt_loc, minlength=nbkt * n_cores)
    cap = int(-(-counts.max() // P))
    tt = nbkt * cap

    starts = np.zeros(nbkt * n_cores + 1, np.int64)
    np.cumsum(counts, out=starts[1:])
    rank = np.arange(len(d_s), dtype=np.int64) - starts[core_of * nbkt + bkt_loc]
    slot = bkt_loc * (cap * P) + rank
    lane = slot % P
    tile = slot // P

    blk_t = np.zeros((n_cores, P, tt), np.int16)       # pad -> block 0
    sm4_t = np.full((n_cores, P, tt), -1.0, np.float32)  # pad -> -1
    dl_t = np.full((n_cores, P, tt), -1.0, np.float32)
    at_t = np.zeros((n_cores, P, tt, 4), np.float32)
    blk_t[core_of, lane, tile] = (s_s >> 2).astype(np.int16)
    sm4_t[core_of, lane, tile] = (s_s & 3).astype(np.float32)
    dl_t[core_of, lane, tile] = (d_s % B).astype(np.float32)
    at_t[core_of, lane, tile] = a_s

    # wrap block ids into the dma_gather index layout: flat index i=t*128+p
    # (within a CT-tile chunk) lives at [i%16, i//16], replicated over the 8
    # groups of 16 partitions.  [C,P,tt] -> [C,128,tt*8] int16.
    CT = CG * cap
    nch = nbkt // CG
    b5 = blk_t.reshape(n_cores, 8, 16, nch, CT)        # p = q*16 + r
    b5 = b5.transpose(0, 3, 2, 4, 1)                   # [C, nch, r, t, q]
    b5 = b5.reshape(n_cores, nch, 16, CT * 8)
    idx16 = np.broadcast_to(b5[:, :, None, :, :],
                            (n_cores, nch, 8, 16, CT * 8))
    idx16 = idx16.transpose(0, 2, 3, 1, 4).reshape(n_cores, 128, tt * 8)
    idx16 = np.ascontiguousarray(idx16)

    cfg = dict(nloc=nloc, nbkt=nbkt, cap=cap, tt=tt, n_pad=n_pad,
               n_cores=n_cores)
    return cfg, idx16, sm4_t, dl_t, at_t


def _householder(a):
    """Orthogonal symmetric involution R with R[:, -1] = a/|a|."""
    h = a.shape[0]
    v = a / np.linalg.norm(a)
    e = np.zeros(h)
    e[h - 1] = 1.0
    u = e - v
    n = np.linalg.norm(u)
    if n < 1e-7:
        return np.eye(h)
    u = u / n
    return np.eye(h) - 2.0 * np.outer(u, u)


# ------------------------------------------------------------- device program
def _build_program(cfg):
    import concourse.bass as bass
    import concourse.mybir as mybir
    import concourse.tile as tile
    from concourse import bacc
    from concourse.masks import make_identity
    from contextlib import ExitStack

    f32 = mybir.dt.float32
    bf16 = mybir.dt.bfloat16
    i16 = mybir.dt.int16
    AT = mybir.AluOpType
    AF = mybir.ActivationFunctionType
    AX = mybir.AxisListType

    nloc, nbkt, cap, tt = cfg["nloc"], cfg["nbkt"], cfg["cap"], cfg["tt"]
    n_pad, n_cores = cfg["n_pad"], cfg["n_cores"]
    anorm = cfg["anorm"]                 # (|c1_att_src|, |c2_att_src|)
    nchunk = nbkt // CG
    CT = CG * cap                    # edge tiles per chunk
    CN = CG * B                      # nodes per chunk (128)
    NCH = nloc // P                  # node-major chunks
    NC_A = nloc // NCA               # f-major chunks
    NB = n_pad // 4                  # 4-node gather blocks
    groups = [list(range(n_cores))]

    nc = bacc.Bacc("TRN2", target_bir_lowering=False, debug=False,
                   num_devices=n_cores)

    # ---------------- external inputs
    xT = nc.dram_tensor("xT", [5, nloc], f32, kind="ExternalInput")
    idx_d = nc.dram_tensor("idx", [P, tt * 8], i16, kind="ExternalInput")
    sm4_d = nc.dram_tensor("sm4", [P, tt], bf16, kind="ExternalInput")
    dl_d = nc.dram_tensor("dl", [P, tt], bf16, kind="ExternalInput")
    at_d = nc.dram_tensor("attr", [P, tt, 4], f32, kind="ExternalInput")
    wnames = [("enc_W", [5, 32]), ("enc_b", [32, 1]),
              ("c1_WY", [32, 32]), ("c1_adY", [32, 1]), ("c1_R", [32, 32]),
              ("c1_adYr", [1, 32]), ("c2_adYr", [1, 32]),
              ("c1_weatte", [1, 4]), ("c1_b", [32, 1]),
              ("c2_WY", [32, 32]), ("c2_adY", [32, 1]),
              ("c2_weatte", [1, 4]),
              ("dec_WT", [1, 128]), ("dec_b", [1, 4])]
    wh = {n: nc.dram_tensor(n, s, f32, kind="ExternalInput")
          for n, s in wnames}
    out_d = nc.dram_tensor("out", [nloc, 4], f32, kind="ExternalOutput")

    # ---------------- internal DRAM
    tab_own = [nc.dram_tensor(f"tab_own{i}", [nloc, 32], bf16)
               for i in range(2)]
    tab_full = [nc.dram_tensor(f"tab_full{i}", [NB, 128], bf16,
                               addr_space="Shared") for i in range(2)]
    num_d = [nc.dram_tensor(f"num{i}", [nloc // P, P, 36], f32)
             for i in range(2)]
    ea2_d = nc.dram_tensor("ea2buf", [P, tt], f32)

    with tile.TileContext(nc) as tc, ExitStack() as ctx:
        const = ctx.enter_context(tc.tile_pool(name="const", bufs=1))
        keep = ctx.enter_context(tc.tile_pool(name="keep", bufs=1))
        nodef = ctx.enter_context(tc.tile_pool(name="nodef", bufs=2))
        psn = ctx.enter_context(tc.tile_pool(name="psn", bufs=1,
                                             space="PSUM"))
        pse = ctx.enter_context(tc.tile_pool(name="pse", bufs=2,
                                             space="PSUM"))
        edge = ctx.enter_context(tc.tile_pool(name="edge", bufs=2))
        nph = ctx.enter_context(tc.tile_pool(name="nph", bufs=1))

        # ---------- constants
        iota_i = const.tile([P, B], mybir.dt.int32)
        nc.gpsimd.iota(iota_i[:], pattern=[[1, B]], base=0,
                       channel_multiplier=0)
        iota16 = const.tile([P, B], bf16)
        nc.vector.tensor_copy(iota16[:], iota_i[:])
        iota4 = const.tile([P, 4], bf16)
        nc.vector.tensor_copy(iota4[:], iota_i[:, 0:4])
        ones_row = const.tile([1, P], f32)
        nc.vector.memset(ones_row[:], 1.0)
        ones16 = const.tile([1, P], bf16)
        nc.vector.memset(ones16[:], 1.0)
        ident = const.tile([P, P], bf16)
        make_identity(nc, ident[:])

        sbw = {}
        for n, s in wnames:
            t = const.tile(s, f32, tag=f"w_{n}")
            nc.sync.dma_start(t[:], wh[n][:])
            sbw[n] = t

        def bcast_row(row_ap, n, out_dt, pool, tag):
            """[1, n] f32 row -> [P, n] tile via PE outer product."""
            ps = psn.tile([P, 512], f32, tag="psb")
            ones = ones16 if row_ap.dtype == bf16 else ones_row
            nc.tensor.matmul(ps[:, 0:n], lhsT=ones[:], rhs=row_ap,
                             start=True, stop=True)
            out = pool.tile([P, n], out_dt, tag=tag)
            nc.scalar.copy(out[:], ps[:, 0:n])
            return out

        we_bc = [bcast_row(sbw["c1_weatte"][:], 4, f32, const, "webc0"),
                 bcast_row(sbw["c2_weatte"][:], 4, f32, const, "webc1")]
        wdall = bcast_row(sbw["dec_WT"][:], 128, f32, const, "wdbc")
        wd_bc = [wdall[:, 32 * k:32 * (k + 1)] for k in range(4)]
        bd_bc = bcast_row(sbw["dec_b"][:], 4, f32, const, "bdbc")

        sbw16 = {}
        for n in ("c1_WY", "c1_adY", "c1_R", "c2_WY", "c2_adY"):
            t16 = const.tile(sbw[n].shape, bf16, tag=f"w16_{n}")
            nc.vector.tensor_copy(t16[:], sbw[n][:])
            sbw16[n] = t16

        # ---------- stage A: encoder (f-major)
        h0T = keep.tile([32, nloc], bf16, tag="hT")
        for c in range(NC_A):
            sl = slice(c * NCA, (c + 1) * NCA)
            xc = nodef.tile([5, NCA], f32, tag="xc")
            nc.sync.dma_start(xc[:], xT[:, sl])
            ps = psn.tile([P, NCA], f32, tag="psn")
            nc.tensor.matmul(ps[0:32, :NCA], lhsT=sbw["enc_W"][:],
                             rhs=xc[:], start=True, stop=True)
            nc.scalar.activation(h0T[:, sl], ps[0:32, :NCA], AF.Relu,
                                 bias=sbw["enc_b"][:], scale=1.0)

        def make_table(hT, wyk, adk, adrk, conv):
            """f-major hidden [32, nloc] -> node-major y1 table + keeps."""
            y1T = keep.tile([32, nloc], bf16, tag="y1T")
            for c in range(NC_A):
                sl = slice(c * NCA, (c + 1) * NCA)
                ps = psn.tile([P, NCA], f32, tag="psn")
                nc.tensor.matmul(ps[0:32, :NCA], lhsT=sbw16[wyk][:],
                                 rhs=hT[:, sl], start=True, stop=True)
                nc.scalar.copy(y1T[:, sl], ps[0:32, :NCA])
            stag = keep.tile([P, NCH, 34], bf16, tag="stag")
            for c in range(NCH):
                ps = psn.tile([P, P], bf16, tag="psnT")
                nc.tensor.transpose(out=ps[:, 0:32],
                                    in_=y1T[:, c * P:(c + 1) * P],
                                    identity=ident[0:32, 0:32])
                nc.scalar.copy(stag[:, c, 0:32], ps[:, 0:32])
            # a_src = |att_src| * y1[:,31]  (node-major, col 32)
            nc.vector.tensor_scalar_mul(stag[:, :, 32], stag[:, :, 31],
                                        float(anorm[conv]))
            # a_dst = y1 @ adY  (node-major, col 33)
            adr_bc = bcast_row(sbw[adrk][:], 32, bf16, keep, "adrbc")
            adt = nph.tile([P, NCH, 32], bf16, tag="adt")
            nc.vector.tensor_tensor(
                out=adt[:], in0=stag[:, :, 0:32],
                in1=adr_bc[:, None, :].to_broadcast([P, NCH, 32]),
                op=AT.mult)
            adf = nph.tile([P, NCH], f32, tag="adf")
            nc.vector.tensor_reduce(out=adf[:], in_=adt[:],
                                    axis=AX.X, op=AT.add)
            nc.vector.tensor_copy(stag[:, :, 33], adf[:])
            # adY broadcast [32, CN] - lhsT for per-chunk a_dst row matmuls
            adY_bc = keep.tile([32, CN], bf16, tag="adYbc")
            nc.vector.tensor_copy(adY_bc[:],
                                  sbw16[adk][:].to_broadcast([32, CN]))
            return y1T, adY_bc, stag

        def publish_table(stag, conv):
            own_view = tab_own[conv][:].rearrange("(c p) r -> p c r", p=P)
            nc.sync.dma_start(own_view, stag[:, :, 0:32])
            nc.gpsimd.collective_compute(
                "AllGather", mybir.AluOpType.bypass,
                replica_groups=groups,
                ins=[tab_own[conv][:]],
                outs=[tab_full[conv][:]],
            )

        y1T1, adY_bc1, stag1 = make_table(h0T, "c1_WY", "c1_adY",
                                          "c1_adYr", 0)
        publish_table(stag1, 0)

        # ---------- edge pipeline
        def edge_pass(conv, y1T, adY_bc):
            NRW = 132 if conv == 0 else 129      # 4*32 quarters + extras
            for ch in range(nchunk):
                tsl = slice(ch * CT, (ch + 1) * CT)
                isl = slice(ch * CT * 8, (ch + 1) * CT * 8)
                idx_s = edge.tile([P, CT * 8], i16, tag="idx")
                nc.sync.dma_start(idx_s[:], idx_d[:, isl])
                dls = edge.tile([P, CT], bf16, tag="dls")
                nc.sync.dma_start(dls[:], dl_d[:, tsl])
                sm4 = edge.tile([P, CT], bf16, tag="sm4")
                nc.sync.dma_start(sm4[:], sm4_d[:, tsl])
                vp4 = edge.tile([P, CT, 128], bf16, tag="v4")
                nc.gpsimd.dma_gather(
                    out_ap=vp4[:], in_ap=tab_full[conv][:],
                    idxs_ap=idx_s[:], num_idxs=CT * P,
                    num_idxs_reg=CT * P, elem_size=128,
                    single_packet=False)
                v4q = vp4[:].rearrange("p t (q k) -> p t q k", q=4)
                oh = edge.tile([P, CT, B], bf16, tag="oh")
                nc.vector.tensor_tensor(
                    out=oh[:], in0=dls[:].to_broadcast([P, CT, B]),
                    in1=iota16[:, None, :].to_broadcast([P, CT, B]),
                    op=AT.is_equal)
                m4 = edge.tile([P, CT, 4], bf16, tag="m4")
                nc.vector.tensor_tensor(
                    out=m4[:], in0=sm4[:].to_broadcast([P, CT, 4]),
                    in1=iota4[:, None, :].to_broadcast([P, CT, 4]),
                    op=AT.is_equal)
                a32ps = psn.tile([P, 512], f32, tag="psb")
                nc.tensor.matmul(a32ps[:, 0:CN], lhsT=adY_bc[:],
                                 rhs=y1T[:, ch * CN:(ch + 1) * CN],
                                 start=True, stop=True)
                a32 = edge.tile([P, CN], bf16, tag="a32")
                nc.scalar.copy(a32[:], a32ps[:, 0:CN])
                dprod = edge.tile([P, CT, B], bf16, tag="dprod")
                a32v = a32[:].rearrange("p (g b) -> p g b", b=B)
                nc.vector.tensor_tensor(
                    out=dprod[:].rearrange("p (g c) b -> p g c b", c=cap),
                    in0=oh[:].rearrange("p (g c) b -> p g c b", c=cap),
                    in1=a32v[:, :, None, :].to_broadcast([P, CG, cap, B]),
                    op=AT.mult)
                dexp = edge.tile([P, CT], f32, tag="dexp")
                nc.vector.tensor_reduce(out=dexp[:], in_=dprod[:],
                                        axis=AX.X, op=AT.add)
                ea = edge.tile([P, CT], f32, tag="ea")
                if conv == 0:
                    at_s = edge.tile([P, CT, 4], f32, tag="at")
                    nc.sync.dma_start(at_s[:], at_d[:, tsl, :])
                    ea4 = edge.tile([P, CT, 4], f32, tag="ea4")
                    nc.vector.tensor_tensor(
                        out=ea4[:], in0=at_s[:],
                        in1=we_bc[0][:, None, :].to_broadcast([P, CT, 4]),
                        op=AT.mult)
                    nc.vector.tensor_reduce(out=ea[:], in_=ea4[:],
                                            axis=AX.X, op=AT.add)
                    ea2 = edge.tile([P, CT], f32, tag="ea2")
                    ea4b = edge.tile([P, CT, 4], f32, tag="ea4")
                    nc.vector.tensor_tensor(
                        out=ea4b[:], in0=at_s[:],
                        in1=we_bc[1][:, None, :].to_broadcast([P, CT, 4]),
                        op=AT.mult)
                    nc.vector.tensor_reduce(out=ea2[:], in_=ea4b[:],
                                            axis=AX.X, op=AT.add)
                    nc.sync.dma_start(ea2_d[:, tsl], ea2[:])
                else:
                    nc.sync.dma_start(ea[:], ea2_d[:, tsl])
                # a_src = |a| * y1[src][31], quarter-selected via m4
                sfq = edge.tile([P, CT, 4], f32, tag="sfq")
                nc.vector.tensor_tensor(out=sfq[:], in0=v4q[:, :, :, 31],
                                        in1=m4[:], op=AT.mult)
                z = edge.tile([P, CT], f32, tag="z")
                nc.vector.tensor_reduce(out=z[:], in_=sfq[:], axis=AX.X,
                                        op=AT.add)
                nc.vector.tensor_scalar_mul(z[:], z[:], float(anorm[conv]))
                nc.vector.tensor_tensor(out=z[:], in0=z[:], in1=ea[:],
                                        op=AT.add)
                nc.vector.tensor_tensor(out=z[:], in0=z[:], in1=dexp[:],
                                        op=AT.add)
                nc.vector.tensor_scalar_min(z[:], z[:], ZCLAMP)
                zn = edge.tile([P, CT], f32, tag="zn")
                nc.vector.tensor_scalar_mul(zn[:], z[:], NEG)
                nc.vector.tensor_tensor(out=z[:], in0=z[:], in1=zn[:],
                                        op=AT.max)
                ex = edge.tile([P, CT], bf16, tag="ex")
                nc.scalar.activation(ex[:], z[:], AF.Exp)
                exq = edge.tile([P, CT, 4], bf16, tag="exq")
                nc.vector.tensor_tensor(
                    out=exq[:], in0=m4[:],
                    in1=ex[:, :, None].to_broadcast([P, CT, 4]),
                    op=AT.mult)
                rhs = edge.tile([P, CT, 132], bf16, tag="rhs")
                nc.vector.tensor_tensor(
                    out=rhs[:, :, 0:128].rearrange("p t (q k) -> p t q k",
                                                   q=4),
                    in0=v4q[:],
                    in1=exq[:, :, :, None].to_broadcast([P, CT, 4, 32]),
                    op=AT.mult)
                nc.vector.tensor_copy(rhs[:, :, 128], ex[:])
                if conv == 0:
                    nc.vector.tensor_copy(rhs[:, :, 129], ea[:])
                    nc.vector.tensor_copy(rhs[:, :, 130], ea2[:])
                    nc.vector.memset(rhs[:, :, 131], 1.0)
                psv = [pse.tile([B, 2 * NRW], f32, tag="pseA", name="pseA"),
                       pse.tile([B, 2 * NRW], f32, tag="pseB", name="pseB")]
                for g in range(CG):
                    ps = psv[g // 2]
                    off = (g % 2) * NRW
                    for i in range(cap):
                        t = g * cap + i
                        nc.tensor.matmul(
                            ps[:, off:off + NRW],
                            lhsT=oh[:, t, :], rhs=rhs[:, t, 0:NRW],
                            start=(i == 0), stop=(i == cap - 1))
                st = edge.tile([B, CG, 36], f32, tag="st")
                NR = 36 if conv == 0 else 33
                for g in range(CG):
                    ps = psv[g // 2]
                    off = (g % 2) * NRW
                    nc.vector.tensor_reduce(
                        out=st[:, g, 0:32],
                        in_=ps[:, off:off + 128].rearrange(
                            "b (q k) -> b k q", q=4),
                        axis=AX.X, op=AT.add)
                    nc.vector.tensor_copy(st[:, g, 32:NR],
                                          ps[:, off + 128:off + NRW])
                nc.sync.dma_start(
                    num_d[conv][ch].rearrange("(g b) r -> b g r", b=B)
                    [:, :, 0:NR],
                    st[:, :, 0:NR])

        edge_pass(0, y1T1, adY_bc1)

        # ---------- node phase
        keepn = keep.tile([P, NCH, 2], f32, tag="keepn")

        def node_finish(conv, stag):
            """-> hy [P, NCH, 32] f32: rotated-basis output, no bias/relu."""
            NR = 36 if conv == 0 else 33
            num = nph.tile([P, NCH, NR], f32, tag="num")
            nc.sync.dma_start(
                num[:],
                num_d[conv][:].rearrange("c p r -> p c r")[:, :, 0:NR])
            if conv == 0:
                easum_ap = num[:, :, 33]
                deg_ap = num[:, :, 35]
                nc.vector.tensor_copy(keepn[:, :, 0], num[:, :, 34])
                nc.vector.tensor_copy(keepn[:, :, 1], num[:, :, 35])
            else:
                easum_ap = keepn[:, :, 0]
                deg_ap = keepn[:, :, 1]
            dg = nph.tile([P, NCH], f32, tag="dg")
            nc.vector.tensor_scalar_max(dg[:], deg_ap, 1.0)
            nc.vector.reciprocal(dg[:], dg[:])
            zl = nph.tile([P, NCH], f32, tag="zl2")
            nc.vector.tensor_tensor(out=zl[:], in0=easum_ap, in1=dg[:],
                                    op=AT.mult)
            asf = nph.tile([P, NCH], f32, tag="asf")
            nc.vector.tensor_copy(asf[:], stag[:, :, 32])
            nc.vector.tensor_tensor(out=zl[:], in0=zl[:], in1=asf[:],
                                    op=AT.add)
            nc.vector.tensor_copy(asf[:], stag[:, :, 33])
            nc.vector.tensor_tensor(out=zl[:], in0=zl[:], in1=asf[:],
                                    op=AT.add)
            zln = nph.tile([P, NCH], f32, tag="zln")
            nc.vector.tensor_scalar_mul(zln[:], zl[:], NEG)
            nc.vector.tensor_tensor(out=zl[:], in0=zl[:], in1=zln[:],
                                    op=AT.max)
            exl = nph.tile([P, NCH], f32, tag="exl")
            nc.scalar.activation(exl[:], zl[:], AF.Exp)
            den = nph.tile([P, NCH], f32, tag="den")
            nc.vector.tensor_tensor(out=den[:], in0=num[:, :, 32],
                                    in1=exl[:], op=AT.add)
            nc.vector.reciprocal(den[:], den[:])
            hy = nph.tile([P, NCH, 32], f32, tag="h")
            exl16 = nph.tile([P, NCH], bf16, tag="exl16")
            nc.vector.tensor_copy(exl16[:], exl[:])
            nc.vector.tensor_tensor(
                out=hy[:], in0=stag[:, :, 0:32],
                in1=exl16[:, :, None].to_broadcast([P, NCH, 32]),
                op=AT.mult)
            nc.vector.tensor_tensor(out=hy[:], in0=hy[:],
                                    in1=num[:, :, 0:32], op=AT.add)
            nc.vector.tensor_tensor(
                out=hy[:], in0=hy[:],
                in1=den[:, :, None].to_broadcast([P, NCH, 32]), op=AT.mult)
            return hy

        hy1 = node_finish(0, stag1)

        # node-major -> f-major chunk-wise; un-rotate h1 = relu(hy1@R1 + b1)
        h1b = nph.tile([P, NCH, 32], bf16, tag="h1b")
        nc.vector.tensor_copy(h1b[:], hy1[:])
        h1T = keep.tile([32, nloc], bf16, tag="hT")
        for c in range(NCH):
            ps = psn.tile([P, P], bf16, tag="psnT")
            nc.tensor.transpose(out=ps[0:32, 0:P], in_=h1b[:, c, :],
                                identity=ident[:])
            hst = nodef.tile([32, P], bf16, tag="hst")
            nc.scalar.copy(hst[:], ps[0:32, 0:P])
            ps2 = psn.tile([P, P], f32, tag="psnR")
            nc.tensor.matmul(ps2[0:32, 0:P], lhsT=sbw16["c1_R"][:],
                             rhs=hst[:], start=True, stop=True)
            nc.scalar.activation(h1T[:, c * P:(c + 1) * P], ps2[0:32, 0:P],
                                 AF.Relu, bias=sbw["c1_b"][:], scale=1.0)

        y1T2, adY_bc2, stag2 = make_table(h1T, "c2_WY", "c2_adY",
                                          "c2_adYr", 1)
        publish_table(stag2, 1)
        edge_pass(1, y1T2, adY_bc2)
        hy2 = node_finish(1, stag2)

        # ---------- decoder + log_softmax (node-major; dec weights carry
        # the conv2 un-rotation and bias fold)
        lg = nph.tile([P, NCH, 4], f32, tag="lg")
        tmp = nph.tile([P, NCH, 32], f32, tag="dtmp")
        for k in range(4):
            nc.vector.tensor_tensor(
                out=tmp[:], in0=hy2[:],
                in1=wd_bc[k][:, None, :].to_broadcast([P, NCH, 32]),
                op=AT.mult)
            nc.vector.tensor_reduce(out=lg[:, :, k], in_=tmp[:], axis=AX.X,
                                    op=AT.add)
        nc.vector.tensor_tensor(
            out=lg[:], in0=lg[:],
            in1=bd_bc[:, None, 0:4].to_broadcast([P, NCH, 4]), op=AT.add)
        mx = nph.tile([P, NCH], f32, tag="mx")
        nc.vector.tensor_reduce(out=mx[:], in_=lg[:], axis=AX.X, op=AT.max)
        nc.vector.tensor_tensor(
            out=lg[:], in0=lg[:],
            in1=mx[:, :, None].to_broadcast([P, NCH, 4]), op=AT.subtract)
        el = nph.tile([P, NCH, 4], f32, tag="el")
        nc.scalar.activation(el[:], lg[:], AF.Exp)
        se = nph.tile([P, NCH], f32, tag="se")
        nc.vector.tensor_reduce(out=se[:], in_=el[:], axis=AX.X, op=AT.add)
        ls = nph.tile([P, NCH], f32, tag="ls")
        nc.scalar.activation(ls[:], se[:], AF.Ln)
        nc.vector.tensor_tensor(
            out=lg[:], in0=lg[:],
            in1=ls[:, :, None].to_broadcast([P, NCH, 4]), op=AT.subtract)
        nc.sync.dma_start(
            out_d[:].rearrange("(c p) r -> p c r", p=P), lg[:])

    nc.compile()
    return nc


_PROGRAM_CACHE = {}


def _get_program(cfg):
    key = (cfg["nloc"], cfg["cap"], cfg["anorm"])
    if key not in _PROGRAM_CACHE:
        _PROGRAM_CACHE[key] = _build_program(cfg)
    return _PROGRAM_CACHE[key]


def _make_in_maps(inputs, cfg, idx16, sm4_t, dl_t, at_t):
    import ml_dtypes
    f32 = np.float32
    x = np.asarray(inputs["x"], f32)
    nloc, n_pad, n_cores = cfg["nloc"], cfg["n_pad"], cfg["n_cores"]
    xp = np.zeros((n_pad, 5), f32)
    xp[:x.shape[0]] = x

    a1 = np.asarray(inputs["c1_att_src"], np.float64)
    a2 = np.asarray(inputs["c2_att_src"], np.float64)
    R1 = _householder(a1)
    R2 = _householder(a2)
    W1 = np.asarray(inputs["c1_W"], np.float64)
    W2 = np.asarray(inputs["c2_W"], np.float64)
    decW = np.asarray(inputs["dec_W"], np.float64)
    wdY = R2 @ decW                                     # [32, 4]
    bY = (np.asarray(inputs["c2_b"], np.float64) @ decW
          + np.asarray(inputs["dec_b"], np.float64))    # [4]

    com = {
        "enc_W": np.asarray(inputs["enc_W"], f32),
        "enc_b": np.asarray(inputs["enc_b"], f32).reshape(32, 1),
        "c1_WY": (W1 @ R1).astype(f32),
        "c1_adY": (R1 @ np.asarray(inputs["c1_att_dst"], np.float64)
                   ).astype(f32).reshape(32, 1),
        "c1_adYr": (R1 @ np.asarray(inputs["c1_att_dst"], np.float64)
                    ).astype(f32).reshape(1, 32),
        "c2_adYr": (R2 @ np.asarray(inputs["c2_att_dst"], np.float64)
                    ).astype(f32).reshape(1, 32),
        "c1_R": R1.astype(f32),
        "c1_weatte": (np.asarray(inputs["c1_We"], f32)
                      @ np.asarray(inputs["c1_att_e"], f32)).reshape(1, 4),
        "c1_b": np.asarray(inputs["c1_b"], f32).reshape(32, 1),
        "c2_WY": (W2 @ R2).astype(f32),
        "c2_adY": (R2 @ np.asarray(inputs["c2_att_dst"], np.float64)
                   ).astype(f32).reshape(32, 1),
        "c2_weatte": (np.asarray(inputs["c2_We"], f32)
                      @ np.asarray(inputs["c2_att_e"], f32)).reshape(1, 4),
        "dec_WT": wdY.T.astype(f32).copy().reshape(1, 128),
        "dec_b": bY.astype(f32).reshape(1, 4),
    }
    in_maps = []
    for c in range(n_cores):
        m = dict(com)
        m["xT"] = xp[c * nloc:(c + 1) * nloc].T.copy()
        m["idx"] = idx16[c]
        m["sm4"] = sm4_t[c].astype(ml_dtypes.bfloat16)
        m["dl"] = dl_t[c].astype(ml_dtypes.bfloat16)
        m["attr"] = at_t[c]
        in_maps.append(m)
    return in_maps


# ------------------------------------------------------------------ entrypoint
def kernel(**inputs):
    ei = np.asarray(inputs["edge_index"])
    attr = np.asarray(inputs["edge_attr"], np.float32)
    n_trucks = int(inputs["num_trucks"])
    n_nodes = np.asarray(inputs["x"]).shape[0]
    n_cores = 8

    src = ei[0].astype(np.int32)
    dst = ei[1].astype(np.int32)
    cfg, idx16, sm4_t, dl_t, at_t = _build_layout(src, dst, attr, n_nodes,
                                                  n_cores)
    cfg["anorm"] = (float(np.linalg.norm(inputs["c1_att_src"])),
                    float(np.linalg.norm(inputs["c2_att_src"])))
    in_maps = _make_in_maps(inputs, cfg, idx16, sm4_t, dl_t, at_t)

    nc = _get_program(cfg)
    from concourse.bass_utils import run_bass_kernel_spmd
    res = run_bass_kernel_spmd(nc, in_maps, core_ids=list(range(n_cores)),
                               trace=False)
    outs = [res.results[c]["out"] for c in range(n_cores)]
    full = np.concatenate(outs, axis=0)[:n_trucks]
    return np.asarray(full, np.float32)


# revision 20
# speedup vs baseline: 1.1144x; 1.1144x over previous
"""Trainium2 Bass kernel for nn_MiningGNN (2-layer GAT message passing).

Sharding: nodes range-sharded across 8 cores; edges sharded by destination
owner (edge-parallel by dst range), grouped into 32-node dst buckets padded
to a uniform tile capacity so one SPMD program serves all cores.

Per conv layer each core computes its shard of the node table y1 = x1 @ R
(R an orthogonal Householder involution whose column 31 is att_src/|att_src|,
so y1[:,31] * |att_src| == a_src), all-gathers it, then streams its edges.
The per-edge gather uses ONE dma_gather per 4-bucket chunk (int16 indices
address 256-byte blocks of 4 consecutive nodes; src>>2 < 32768 fits int16),
instead of one indirect DMA per 128-edge tile — this removes the ~1us/tile
SWDGE fixed cost that dominated the per-tile version. The unneeded 3 nodes
per gathered block are masked by folding a per-quarter mask into ex: the
scatter one-hot matmul accumulates a 128-wide (4 quarters x 32 features)
numerator that is quarter-folded to 32 right out of PSUM. Aggregation stays
in the rotated basis; the un-rotation (h = hy @ R) is applied f-major fused
with bias+relu for conv1 and folded into the decoder weights for conv2.
segment_max is skipped (softmax is invariant; scores are tiny and clamped).
"""
import numpy as np

P = 128          # partitions / edge-tile height
B = 32           # dst-bucket width (nodes)
CG = 4           # buckets per edge-pipeline chunk
NCA = 448        # f-major node chunk (divides nloc)
NEG = 0.2        # leaky_relu slope
ZCLAMP = 30.0


# ----------------------------------------------------------------- host layout
def _build_layout(src, dst, attr, n_nodes, n_cores):
    """Sort edges by dst, shard by dst range, bucket and pad to uniform tiles."""
    nloc_raw = -(-n_nodes // n_cores)
    nbkt = -(-nloc_raw // B)
    nbkt = -(-nbkt // CG) * CG                         # multiple of CG
    while (nbkt * B) % NCA:
        nbkt += CG
    nloc = nbkt * B
    n_pad = nloc * n_cores

    order = np.argsort(dst, kind="stable")
    s_s = src[order]
    d_s = dst[order]
    a_s = attr[order]
    core_of = d_s // nloc
    gbkt = d_s // B
    bkt_loc = gbkt - core_of * nbkt
    counts = np.bincount(core_of * nbkt + bkt_loc, minlength=nbkt * n_cores)
    cap = int(-(-counts.max() // P))
    tt = nbkt * cap

    starts = np.zeros(nbkt * n_cores + 1, np.int64)
    np.cumsum(counts, out=starts[1:])
    rank = np.arange(len(d_s), dtype=np.int64) - starts[core_of * nbkt + bkt_loc]
    slot = bkt_loc * (cap * P) + rank
    lane = slot % P
    tile = slot // P

    blk_t = np.zeros((n_cores, P, tt), np.int16)       # pad -> block 0
    sm4_t = np.full((n_cores, P, tt), -1.0, np.float32)  # pad -> -1
    dl_t = np.full((n_cores, P, tt), -1.0, np.float32)
    at_t = np.zeros((n_cores, P, tt, 4), np.float32)
    blk_t[core_of, lane, tile] = (s_s >> 2).astype(np.int16)
    sm4_t[core_of, lane, tile] = (s_s & 3).astype(np.float32)
    dl_t[core_of, lane, tile] = (d_s % B).astype(np.float32)
    at_t[core_of, lane, tile] = a_s

    # wrap block ids into the dma_gather index layout: flat index i=t*128+p
    # (within a CT-tile chunk) lives at [i%16, i//16], replicated over the 8
    # groups of 16 partitions.  [C,P,tt] -> [C,128,tt*8] int16.
    CT = CG * cap
    nch = nbkt // CG
    b5 = blk_t.reshape(n_cores, 8, 16, nch, CT)        # p = q*16 + r
    b5 = b5.transpose(0, 3, 2, 4, 1)                   # [C, nch, r, t, q]
    b5 = b5.reshape(n_cores, nch, 16, CT * 8)
    idx16 = np.broadcast_to(b5[:, :, None, :, :],
                            (n_cores, nch, 8, 16, CT * 8))
    idx16 = idx16.transpose(0, 2, 3, 1, 4).reshape(n_cores, 128, tt * 8)
    idx16 = np.ascontiguousarray(idx16)

    cfg = dict(nloc=nloc, nbkt=nbkt, cap=cap, tt=tt, n_pad=n_pad,
               n_cores=n_cores)
    return cfg, idx16, sm4_t, dl_t, at_t


def _householder(a):
    """Orthogonal symmetric involution R with R[:, -1] = a/|a|."""
    h = a.shape[0]
    v = a / np.linalg.norm(a)
    e = np.zeros(h)
    e[h - 1] = 1.0
    u = e - v
    n = np.linalg.norm(u)
    if n < 1e-7:
        return np.eye(h)
    u = u / n
    return np.eye(h) - 2.0 * np.outer(u, u)


# ------------------------------------------------------------- device program
def _build_program(cfg):
    import concourse.bass as bass
    import concourse.mybir as mybir
    import concourse.tile as tile
    from concourse import bacc
    from concourse.masks import make_identity
    from contextlib import ExitStack

    f32 = mybir.dt.float32
    bf16 = mybir.dt.bfloat16
    i16 = mybir.dt.int16
    AT = mybir.AluOpType
    AF = mybir.ActivationFunctionType
    AX = mybir.AxisListType

    nloc, nbkt, cap, tt = cfg["nloc"], cfg["nbkt"], cfg["cap"], cfg["tt"]
    n_pad, n_cores = cfg["n_pad"], cfg["n_cores"]
    anorm = cfg["anorm"]                 # (|c1_att_src|, |c2_att_src|)
    nchunk = nbkt // CG
    CT = CG * cap                    # edge tiles per chunk
    CN = CG * B                      # nodes per chunk (128)
    NCH = nloc // P                  # node-major chunks
    NC_A = nloc // NCA               # f-major chunks
    NB = n_pad // 4                  # 4-node gather blocks
    groups = [list(range(n_cores))]

    nc = bacc.Bacc("TRN2", target_bir_lowering=False, debug=False,
                   num_devices=n_cores)

    # ---------------- external inputs
    xT = nc.dram_tensor("xT", [5, nloc], f32, kind="ExternalInput")
    idx_d = nc.dram_tensor("idx", [P, tt * 8], i16, kind="ExternalInput")
    sm4_d = nc.dram_tensor("sm4", [P, tt], bf16, kind="ExternalInput")
    dl_d = nc.dram_tensor("dl", [P, tt], bf16, kind="ExternalInput")
    at_d = nc.dram_tensor("attr", [P, tt, 4], f32, kind="ExternalInput")
    wnames = [("enc_W", [5, 32]), ("enc_b", [32, 1]),
              ("c1_WY", [32, 32]), ("c1_adY", [32, 1]), ("c1_R", [32, 32]),
              ("c1_adYr", [1, 32]), ("c2_adYr", [1, 32]),
              ("c1_weatte", [1, 4]), ("c1_b", [32, 1]),
              ("c2_WY", [32, 32]), ("c2_adY", [32, 1]),
              ("c2_weatte", [1, 4]),
              ("dec_WT", [1, 128]), ("dec_b", [1, 4])]
    wh = {n: nc.dram_tensor(n, s, f32, kind="ExternalInput")
          for n, s in wnames}
    out_d = nc.dram_tensor("out", [nloc, 4], f32, kind="ExternalOutput")

    # ---------------- internal DRAM
    tab_own = [nc.dram_tensor(f"tab_own{i}", [nloc, 32], bf16)
               for i in range(2)]
    tab_full = [nc.dram_tensor(f"tab_full{i}", [NB, 128], bf16,
                               addr_space="Shared") for i in range(2)]
    num_d = [nc.dram_tensor(f"num{i}", [nloc // P, P, 36], f32)
             for i in range(2)]
    ea2_d = nc.dram_tensor("ea2buf", [P, tt], f32)

    with tile.TileContext(nc) as tc, ExitStack() as ctx:
        const = ctx.enter_context(tc.tile_pool(name="const", bufs=1))
        keep = ctx.enter_context(tc.tile_pool(name="keep", bufs=1))
        nodef = ctx.enter_context(tc.tile_pool(name="nodef", bufs=2))
        psn = ctx.enter_context(tc.tile_pool(name="psn", bufs=1,
                                             space="PSUM"))
        pse = ctx.enter_context(tc.tile_pool(name="pse", bufs=2,
                                             space="PSUM"))
        edge = ctx.enter_context(tc.tile_pool(name="edge", bufs=2))
        nph = ctx.enter_context(tc.tile_pool(name="nph", bufs=1))

        # ---------- constants
        iota_i = const.tile([P, B], mybir.dt.int32)
        nc.gpsimd.iota(iota_i[:], pattern=[[1, B]], base=0,
                       channel_multiplier=0)
        iota16 = const.tile([P, B], bf16)
        nc.vector.tensor_copy(iota16[:], iota_i[:])
        iota4 = const.tile([P, 4], bf16)
        nc.vector.tensor_copy(iota4[:], iota_i[:, 0:4])
        ones_row = const.tile([1, P], f32)
        nc.vector.memset(ones_row[:], 1.0)
        ones16 = const.tile([1, P], bf16)
        nc.vector.memset(ones16[:], 1.0)
        ident = const.tile([P, P], bf16)
        make_identity(nc, ident[:])

        sbw = {}
        for n, s in wnames:
            t = const.tile(s, f32, tag=f"w_{n}")
            nc.sync.dma_start(t[:], wh[n][:])
            sbw[n] = t

        def bcast_row(row_ap, n, out_dt, pool, tag):
            """[1, n] f32 row -> [P, n] tile via PE outer product."""
            ps = psn.tile([P, 512], f32, tag="psb")
            ones = ones16 if row_ap.dtype == bf16 else ones_row
            nc.tensor.matmul(ps[:, 0:n], lhsT=ones[:], rhs=row_ap,
                             start=True, stop=True)
            out = pool.tile([P, n], out_dt, tag=tag)
            nc.scalar.copy(out[:], ps[:, 0:n])
            return out

        we_bc = [bcast_row(sbw["c1_weatte"][:], 4, f32, const, "webc0"),
                 bcast_row(sbw["c2_weatte"][:], 4, f32, const, "webc1")]
        wdall = bcast_row(sbw["dec_WT"][:], 128, f32, const, "wdbc")
        wd_bc = [wdall[:, 32 * k:32 * (k + 1)] for k in range(4)]
        bd_bc = bcast_row(sbw["dec_b"][:], 4, f32, const, "bdbc")

        sbw16 = {}
        for n in ("c1_WY", "c1_adY", "c1_R", "c2_WY", "c2_adY"):
            t16 = const.tile(sbw[n].shape, bf16, tag=f"w16_{n}")
            nc.vector.tensor_copy(t16[:], sbw[n][:])
            sbw16[n] = t16

        # ---------- stage A: encoder (f-major)
        h0T = keep.tile([32, nloc], bf16, tag="hT")
        for c in range(NC_A):
            sl = slice(c * NCA, (c + 1) * NCA)
            xc = nodef.tile([5, NCA], f32, tag="xc")
            nc.sync.dma_start(xc[:], xT[:, sl])
            ps = psn.tile([P, NCA], f32, tag="psn")
            nc.tensor.matmul(ps[0:32, :NCA], lhsT=sbw["enc_W"][:],
                             rhs=xc[:], start=True, stop=True)
            nc.scalar.activation(h0T[:, sl], ps[0:32, :NCA], AF.Relu,
                                 bias=sbw["enc_b"][:], scale=1.0)

        def make_table(hT, wyk, adk, adrk, conv):
            """f-major hidden [32, nloc] -> node-major y1 table + keeps."""
            y1T = keep.tile([32, nloc], bf16, tag="y1T")
            for c in range(NC_A):
                sl = slice(c * NCA, (c + 1) * NCA)
                ps = psn.tile([P, NCA], f32, tag="psn")
                nc.tensor.matmul(ps[0:32, :NCA], lhsT=sbw16[wyk][:],
                                 rhs=hT[:, sl], start=True, stop=True)
                nc.scalar.copy(y1T[:, sl], ps[0:32, :NCA])
            stag = keep.tile([P, NCH, 34], bf16, tag="stag")
            for c in range(NCH):
                ps = psn.tile([P, P], bf16, tag="psnT")
                nc.tensor.transpose(out=ps[:, 0:32],
                                    in_=y1T[:, c * P:(c + 1) * P],
                                    identity=ident[0:32, 0:32])
                nc.scalar.copy(stag[:, c, 0:32], ps[:, 0:32])
            # a_src = |att_src| * y1[:,31]  (node-major, col 32)
            nc.vector.tensor_scalar_mul(stag[:, :, 32], stag[:, :, 31],
                                        float(anorm[conv]))
            # a_dst = y1 @ adY  (node-major, col 33)
            adr_bc = bcast_row(sbw[adrk][:], 32, bf16, keep, "adrbc")
            adt = nph.tile([P, NCH, 32], bf16, tag="adt")
            nc.vector.tensor_tensor(
                out=adt[:], in0=stag[:, :, 0:32],
                in1=adr_bc[:, None, :].to_broadcast([P, NCH, 32]),
                op=AT.mult)
            adf = nph.tile([P, NCH], f32, tag="adf")
            nc.vector.tensor_reduce(out=adf[:], in_=adt[:],
                                    axis=AX.X, op=AT.add)
            nc.vector.tensor_copy(stag[:, :, 33], adf[:])
            # adY broadcast [32, CN] - lhsT for per-chunk a_dst row matmuls
            adY_bc = keep.tile([32, CN], bf16, tag="adYbc")
            nc.vector.tensor_copy(adY_bc[:],
                                  sbw16[adk][:].to_broadcast([32, CN]))
            return y1T, adY_bc, stag

        def publish_table(stag, conv):
            own_view = tab_own[conv][:].rearrange("(c p) r -> p c r", p=P)
            nc.sync.dma_start(own_view, stag[:, :, 0:32])
            nc.gpsimd.collective_compute(
                "AllGather", mybir.AluOpType.bypass,
                replica_groups=groups,
                ins=[tab_own[conv][:]],
                outs=[tab_full[conv][:]],
            )

        y1T1, adY_bc1, stag1 = make_table(h0T, "c1_WY", "c1_adY",
                                          "c1_adYr", 0)
        publish_table(stag1, 0)

        # ---------- edge pipeline
        def edge_pass(conv, y1T, adY_bc):
            NRW = 132 if conv == 0 else 129      # 4*32 quarters + extras
            for ch in range(nchunk):
                tsl = slice(ch * CT, (ch + 1) * CT)
                isl = slice(ch * CT * 8, (ch + 1) * CT * 8)
                idx_s = edge.tile([P, CT * 8], i16, tag="idx")
                nc.sync.dma_start(idx_s[:], idx_d[:, isl])
                dls = edge.tile([P, CT], bf16, tag="dls")
                nc.sync.dma_start(dls[:], dl_d[:, tsl])
                sm4 = edge.tile([P, CT], bf16, tag="sm4")
                nc.sync.dma_start(sm4[:], sm4_d[:, tsl])
                vp4 = edge.tile([P, CT, 128], bf16, tag="v4", bufs=3)
                nc.gpsimd.dma_gather(
                    out_ap=vp4[:], in_ap=tab_full[conv][:],
                    idxs_ap=idx_s[:], num_idxs=CT * P,
                    num_idxs_reg=CT * P, elem_size=128,
                    single_packet=False)
                v4q = vp4[:].rearrange("p t (q k) -> p t q k", q=4)
                oh = edge.tile([P, CT, B], bf16, tag="oh")
                nc.vector.tensor_tensor(
                    out=oh[:], in0=dls[:].to_broadcast([P, CT, B]),
                    in1=iota16[:, None, :].to_broadcast([P, CT, B]),
                    op=AT.is_equal)
                m4 = edge.tile([P, CT, 4], bf16, tag="m4")
                nc.vector.tensor_tensor(
                    out=m4[:], in0=sm4[:].to_broadcast([P, CT, 4]),
                    in1=iota4[:, None, :].to_broadcast([P, CT, 4]),
                    op=AT.is_equal)
                a32ps = psn.tile([P, 512], f32, tag="psb")
                nc.tensor.matmul(a32ps[:, 0:CN], lhsT=adY_bc[:],
                                 rhs=y1T[:, ch * CN:(ch + 1) * CN],
                                 start=True, stop=True)
                a32 = edge.tile([P, CN], bf16, tag="a32")
                nc.scalar.copy(a32[:], a32ps[:, 0:CN])
                dprod = edge.tile([P, CT, B], bf16, tag="dprod")
                a32v = a32[:].rearrange("p (g b) -> p g b", b=B)
                nc.vector.tensor_tensor(
                    out=dprod[:].rearrange("p (g c) b -> p g c b", c=cap),
                    in0=oh[:].rearrange("p (g c) b -> p g c b", c=cap),
                    in1=a32v[:, :, None, :].to_broadcast([P, CG, cap, B]),
                    op=AT.mult)
                dexp = edge.tile([P, CT], f32, tag="dexp")
                nc.vector.tensor_reduce(out=dexp[:], in_=dprod[:],
                                        axis=AX.X, op=AT.add)
                ea = edge.tile([P, CT], f32, tag="ea")
                if conv == 0:
                    at_s = edge.tile([P, CT, 4], f32, tag="at")
                    nc.sync.dma_start(at_s[:], at_d[:, tsl, :])
                    ea4 = edge.tile([P, CT, 4], f32, tag="ea4")
                    nc.vector.tensor_tensor(
                        out=ea4[:], in0=at_s[:],
                        in1=we_bc[0][:, None, :].to_broadcast([P, CT, 4]),
                        op=AT.mult)
                    nc.vector.tensor_reduce(out=ea[:], in_=ea4[:],
                                            axis=AX.X, op=AT.add)
                    ea2 = edge.tile([P, CT], f32, tag="ea2")
                    ea4b = edge.tile([P, CT, 4], f32, tag="ea4")
                    nc.vector.tensor_tensor(
                        out=ea4b[:], in0=at_s[:],
                        in1=we_bc[1][:, None, :].to_broadcast([P, CT, 4]),
                        op=AT.mult)
                    nc.vector.tensor_reduce(out=ea2[:], in_=ea4b[:],
                                            axis=AX.X, op=AT.add)
                    nc.sync.dma_start(ea2_d[:, tsl], ea2[:])
                else:
                    nc.sync.dma_start(ea[:], ea2_d[:, tsl])
                # a_src = |a| * y1[src][31], quarter-selected via m4
                sfq = edge.tile([P, CT, 4], f32, tag="sfq")
                nc.vector.tensor_tensor(out=sfq[:], in0=v4q[:, :, :, 31],
                                        in1=m4[:], op=AT.mult)
                z = edge.tile([P, CT], f32, tag="z")
                nc.vector.tensor_reduce(out=z[:], in_=sfq[:], axis=AX.X,
                                        op=AT.add)
                nc.vector.tensor_scalar_mul(z[:], z[:], float(anorm[conv]))
                nc.vector.tensor_tensor(out=z[:], in0=z[:], in1=ea[:],
                                        op=AT.add)
                nc.vector.tensor_tensor(out=z[:], in0=z[:], in1=dexp[:],
                                        op=AT.add)
                nc.vector.tensor_scalar_min(z[:], z[:], ZCLAMP)
                zn = edge.tile([P, CT], f32, tag="zn")
                nc.vector.tensor_scalar_mul(zn[:], z[:], NEG)
                nc.vector.tensor_tensor(out=z[:], in0=z[:], in1=zn[:],
                                        op=AT.max)
                ex = edge.tile([P, CT], bf16, tag="ex")
                nc.scalar.activation(ex[:], z[:], AF.Exp)
                exq = edge.tile([P, CT, 4], bf16, tag="exq")
                nc.vector.tensor_tensor(
                    out=exq[:], in0=m4[:],
                    in1=ex[:, :, None].to_broadcast([P, CT, 4]),
                    op=AT.mult)
                rhs = edge.tile([P, CT, 128], bf16, tag="rhs")
                nc.vector.tensor_tensor(
                    out=rhs[:].rearrange("p t (q k) -> p t q k", q=4),
                    in0=v4q[:],
                    in1=exq[:, :, :, None].to_broadcast([P, CT, 4, 32]),
                    op=AT.mult)
                ext = edge.tile([P, CT, 4], bf16, tag="ext")
                NE = NRW - 128
                nc.vector.tensor_copy(ext[:, :, 0], ex[:])
                if conv == 0:
                    nc.vector.tensor_copy(ext[:, :, 1], ea[:])
                    nc.vector.tensor_copy(ext[:, :, 2], ea2[:])
                    nc.vector.memset(ext[:, :, 3], 1.0)
                psv = [pse.tile([B, 2 * NRW], f32, tag="pseA", name="pseA"),
                       pse.tile([B, 2 * NRW], f32, tag="pseB", name="pseB")]
                for g in range(CG):
                    ps = psv[g // 2]
                    off = (g % 2) * NRW
                    for i in range(cap):
                        t = g * cap + i
                        nc.tensor.matmul(
                            ps[:, off:off + 128],
                            lhsT=oh[:, t, :], rhs=rhs[:, t, :],
                            start=(i == 0), stop=(i == cap - 1))
                        nc.tensor.matmul(
                            ps[:, off + 128:off + NRW],
                            lhsT=oh[:, t, :], rhs=ext[:, t, 0:NE],
                            start=(i == 0), stop=(i == cap - 1))
                st = edge.tile([B, CG, 36], f32, tag="st")
                NR = 36 if conv == 0 else 33
                for g in range(CG):
                    ps = psv[g // 2]
                    off = (g % 2) * NRW
                    nc.vector.tensor_reduce(
                        out=st[:, g, 0:32],
                        in_=ps[:, off:off + 128].rearrange(
                            "b (q k) -> b k q", q=4),
                        axis=AX.X, op=AT.add)
                    nc.vector.tensor_copy(st[:, g, 32:NR],
                                          ps[:, off + 128:off + NRW])
                nc.sync.dma_start(
                    num_d[conv][ch].rearrange("(g b) r -> b g r", b=B)
                    [:, :, 0:NR],
                    st[:, :, 0:NR])

        edge_pass(0, y1T1, adY_bc1)

        # ---------- node phase
        keepn = keep.tile([P, NCH, 2], f32, tag="keepn")

        def node_finish(conv, stag):
            """-> hy [P, NCH, 32] f32: rotated-basis output, no bias/relu."""
            NR = 36 if conv == 0 else 33
            num = nph.tile([P, NCH, NR], f32, tag="num")
            nc.sync.dma_start(
                num[:],
                num_d[conv][:].rearrange("c p r -> p c r")[:, :, 0:NR])
            if conv == 0:
                easum_ap = num[:, :, 33]
                deg_ap = num[:, :, 35]
                nc.vector.tensor_copy(keepn[:, :, 0], num[:, :, 34])
                nc.vector.tensor_copy(keepn[:, :, 1], num[:, :, 35])
            else:
                easum_ap = keepn[:, :, 0]
                deg_ap = keepn[:, :, 1]
            dg = nph.tile([P, NCH], f32, tag="dg")
            nc.vector.tensor_scalar_max(dg[:], deg_ap, 1.0)
            nc.vector.reciprocal(dg[:], dg[:])
            zl = nph.tile([P, NCH], f32, tag="zl2")
            nc.vector.tensor_tensor(out=zl[:], in0=easum_ap, in1=dg[:],
                                    op=AT.mult)
            asf = nph.tile([P, NCH], f32, tag="asf")
            nc.vector.tensor_copy(asf[:], stag[:, :, 32])
            nc.vector.tensor_tensor(out=zl[:], in0=zl[:], in1=asf[:],
                                    op=AT.add)
            nc.vector.tensor_copy(asf[:], stag[:, :, 33])
            nc.vector.tensor_tensor(out=zl[:], in0=zl[:], in1=asf[:],
                                    op=AT.add)
            zln = nph.tile([P, NCH], f32, tag="zln")
            nc.vector.tensor_scalar_mul(zln[:], zl[:], NEG)
            nc.vector.tensor_tensor(out=zl[:], in0=zl[:], in1=zln[:],
                                    op=AT.max)
            exl = nph.tile([P, NCH], f32, tag="exl")
            nc.scalar.activation(exl[:], zl[:], AF.Exp)
            den = nph.tile([P, NCH], f32, tag="den")
            nc.vector.tensor_tensor(out=den[:], in0=num[:, :, 32],
                                    in1=exl[:], op=AT.add)
            nc.vector.reciprocal(den[:], den[:])
            hy = nph.tile([P, NCH, 32], f32, tag="h")
            exl16 = nph.tile([P, NCH], bf16, tag="exl16")
            nc.vector.tensor_copy(exl16[:], exl[:])
            nc.vector.tensor_tensor(
                out=hy[:], in0=stag[:, :, 0:32],
                in1=exl16[:, :, None].to_broadcast([P, NCH, 32]),
                op=AT.mult)
            nc.vector.tensor_tensor(out=hy[:], in0=hy[:],
                                    in1=num[:, :, 0:32], op=AT.add)
            nc.vector.tensor_tensor(
                out=hy[:], in0=hy[:],
                in1=den[:, :, None].to_broadcast([P, NCH, 32]), op=AT.mult)
            return hy

        hy1 = node_finish(0, stag1)

        # node-major -> f-major chunk-wise; un-rotate h1 = relu(hy1@R1 + b1)
        h1b = nph.tile([P, NCH, 32], bf16, tag="h1b")
        nc.vector.tensor_copy(h1b[:], hy1[:])
        h1T = keep.tile([32, nloc], bf16, tag="hT")
        for c in range(NCH):
            ps = psn.tile([P, P], bf16, tag="psnT")
            nc.tensor.transpose(out=ps[0:32, 0:P], in_=h1b[:, c, :],
                                identity=ident[:])
            hst = nodef.tile([32, P], bf16, tag="hst")
            nc.scalar.copy(hst[:], ps[0:32, 0:P])
            ps2 = psn.tile([P, P], f32, tag="psnR")
            nc.tensor.matmul(ps2[0:32, 0:P], lhsT=sbw16["c1_R"][:],
                             rhs=hst[:], start=True, stop=True)
            nc.scalar.activation(h1T[:, c * P:(c + 1) * P], ps2[0:32, 0:P],
                                 AF.Relu, bias=sbw["c1_b"][:], scale=1.0)

        y1T2, adY_bc2, stag2 = make_table(h1T, "c2_WY", "c2_adY",
                                          "c2_adYr", 1)
        publish_table(stag2, 1)
        edge_pass(1, y1T2, adY_bc2)
        hy2 = node_finish(1, stag2)

        # ---------- decoder + log_softmax (node-major; dec weights carry
        # the conv2 un-rotation and bias fold)
        lg = nph.tile([P, NCH, 4], f32, tag="lg")
        tmp = nph.tile([P, NCH, 32], f32, tag="dtmp")
        for k in range(4):
            nc.vector.tensor_tensor(
                out=tmp[:], in0=hy2[:],
                in1=wd_bc[k][:, None, :].to_broadcast([P, NCH, 32]),
                op=AT.mult)
            nc.vector.tensor_reduce(out=lg[:, :, k], in_=tmp[:], axis=AX.X,
                                    op=AT.add)
        nc.vector.tensor_tensor(
            out=lg[:], in0=lg[:],
            in1=bd_bc[:, None, 0:4].to_broadcast([P, NCH, 4]), op=AT.add)
        mx = nph.tile([P, NCH], f32, tag="mx")
        nc.vector.tensor_reduce(out=mx[:], in_=lg[:], axis=AX.X, op=AT.max)
        nc.vector.tensor_tensor(
            out=lg[:], in0=lg[:],
            in1=mx[:, :, None].to_broadcast([P, NCH, 4]), op=AT.subtract)
        el = nph.tile([P, NCH, 4], f32, tag="el")
        nc.scalar.activation(el[:], lg[:], AF.Exp)
        se = nph.tile([P, NCH], f32, tag="se")
        nc.vector.tensor_reduce(out=se[:], in_=el[:], axis=AX.X, op=AT.add)
        ls = nph.tile([P, NCH], f32, tag="ls")
        nc.scalar.activation(ls[:], se[:], AF.Ln)
        nc.vector.tensor_tensor(
            out=lg[:], in0=lg[:],
            in1=ls[:, :, None].to_broadcast([P, NCH, 4]), op=AT.subtract)
        nc.sync.dma_start(
            out_d[:].rearrange("(c p) r -> p c r", p=P), lg[:])

    nc.compile()
    return nc


_PROGRAM_CACHE = {}


def _get_program(cfg):
    key = (cfg["nloc"], cfg["cap"], cfg["anorm"])
    if key not in _PROGRAM_CACHE:
        _PROGRAM_CACHE[key] = _build_program(cfg)
    return _PROGRAM_CACHE[key]


def _make_in_maps(inputs, cfg, idx16, sm4_t, dl_t, at_t):
    import ml_dtypes
    f32 = np.float32
    x = np.asarray(inputs["x"], f32)
    nloc, n_pad, n_cores = cfg["nloc"], cfg["n_pad"], cfg["n_cores"]
    xp = np.zeros((n_pad, 5), f32)
    xp[:x.shape[0]] = x

    a1 = np.asarray(inputs["c1_att_src"], np.float64)
    a2 = np.asarray(inputs["c2_att_src"], np.float64)
    R1 = _householder(a1)
    R2 = _householder(a2)
    W1 = np.asarray(inputs["c1_W"], np.float64)
    W2 = np.asarray(inputs["c2_W"], np.float64)
    decW = np.asarray(inputs["dec_W"], np.float64)
    wdY = R2 @ decW                                     # [32, 4]
    bY = (np.asarray(inputs["c2_b"], np.float64) @ decW
          + np.asarray(inputs["dec_b"], np.float64))    # [4]

    com = {
        "enc_W": np.asarray(inputs["enc_W"], f32),
        "enc_b": np.asarray(inputs["enc_b"], f32).reshape(32, 1),
        "c1_WY": (W1 @ R1).astype(f32),
        "c1_adY": (R1 @ np.asarray(inputs["c1_att_dst"], np.float64)
                   ).astype(f32).reshape(32, 1),
        "c1_adYr": (R1 @ np.asarray(inputs["c1_att_dst"], np.float64)
                    ).astype(f32).reshape(1, 32),
        "c2_adYr": (R2 @ np.asarray(inputs["c2_att_dst"], np.float64)
                    ).astype(f32).reshape(1, 32),
        "c1_R": R1.astype(f32),
        "c1_weatte": (np.asarray(inputs["c1_We"], f32)
                      @ np.asarray(inputs["c1_att_e"], f32)).reshape(1, 4),
        "c1_b": np.asarray(inputs["c1_b"], f32).reshape(32, 1),
        "c2_WY": (W2 @ R2).astype(f32),
        "c2_adY": (R2 @ np.asarray(inputs["c2_att_dst"], np.float64)
                   ).astype(f32).reshape(32, 1),
        "c2_weatte": (np.asarray(inputs["c2_We"], f32)
                      @ np.asarray(inputs["c2_att_e"], f32)).reshape(1, 4),
        "dec_WT": wdY.T.astype(f32).copy().reshape(1, 128),
        "dec_b": bY.astype(f32).reshape(1, 4),
    }
    in_maps = []
    for c in range(n_cores):
        m = dict(com)
        m["xT"] = xp[c * nloc:(c + 1) * nloc].T.copy()
        m["idx"] = idx16[c]
        m["sm4"] = sm4_t[c].astype(ml_dtypes.bfloat16)
        m["dl"] = dl_t[c].astype(ml_dtypes.bfloat16)
        m["attr"] = at_t[c]
        in_maps.append(m)
    return in_maps


# ------------------------------------------------------------------ entrypoint
def kernel(**inputs):
    ei = np.asarray(inputs["edge_index"])
    attr = np.asarray(inputs["edge_attr"], np.float32)
    n_trucks = int(inputs["num_trucks"])
    n_nodes = np.asarray(inputs["x"]).shape[0]
    n_cores = 8

    src = ei[0].astype(np.int32)
    dst = ei[1].astype(np.int32)
    cfg, idx16, sm4_t, dl_t, at_t = _build_layout(src, dst, attr, n_nodes,
                                                  n_cores)
    cfg["anorm"] = (float(np.linalg.norm(inputs["c1_att_src"])),
                    float(np.linalg.norm(inputs["c2_att_src"])))
    in_maps = _make_in_maps(inputs, cfg, idx16, sm4_t, dl_t, at_t)

    nc = _get_program(cfg)
    from concourse.bass_utils import run_bass_kernel_spmd
    res = run_bass_kernel_spmd(nc, in_maps, core_ids=list(range(n_cores)),
                               trace=False)
    outs = [res.results[c]["out"] for c in range(n_cores)]
    full = np.concatenate(outs, axis=0)[:n_trucks]
    return np.asarray(full, np.float32)


# revision 21
# speedup vs baseline: 1.1410x; 1.0239x over previous
"""Trainium2 Bass kernel for nn_MiningGNN (2-layer GAT message passing).

Sharding: nodes range-sharded across 8 cores; edges sharded by destination
owner (edge-parallel by dst range), grouped into 32-node dst buckets padded
to a uniform tile capacity so one SPMD program serves all cores.

Per conv layer each core computes its shard of the node table y1 = x1 @ R
(R an orthogonal Householder involution whose column 31 is att_src/|att_src|,
so y1[:,31] * |att_src| == a_src), all-gathers it, then streams its edges.
The per-edge gather uses ONE dma_gather per 4-bucket chunk (int16 indices
address 256-byte blocks of 4 consecutive nodes; src>>2 < 32768 fits int16),
instead of one indirect DMA per 128-edge tile — this removes the ~1us/tile
SWDGE fixed cost that dominated the per-tile version. The unneeded 3 nodes
per gathered block are masked by folding a per-quarter mask into ex: the
scatter one-hot matmul accumulates a 128-wide (4 quarters x 32 features)
numerator that is quarter-folded to 32 right out of PSUM. Aggregation stays
in the rotated basis; the un-rotation (h = hy @ R) is applied f-major fused
with bias+relu for conv1 and folded into the decoder weights for conv2.
segment_max is skipped (softmax is invariant; scores are tiny and clamped).
"""
import numpy as np

P = 128          # partitions / edge-tile height
B = 32           # dst-bucket width (nodes)
CG = 4           # buckets per edge-pipeline chunk
NCA = 448        # f-major node chunk (divides nloc)
NEG = 0.2        # leaky_relu slope
ZCLAMP = 30.0


# ----------------------------------------------------------------- host layout
def _build_layout(src, dst, attr, n_nodes, n_cores):
    """Sort edges by dst, shard by dst range, bucket and pad to uniform tiles."""
    nloc_raw = -(-n_nodes // n_cores)
    nbkt = -(-nloc_raw // B)
    nbkt = -(-nbkt // CG) * CG                         # multiple of CG
    while (nbkt * B) % NCA:
        nbkt += CG
    nloc = nbkt * B
    n_pad = nloc * n_cores

    order = np.argsort(dst, kind="stable")
    s_s = src[order]
    d_s = dst[order]
    a_s = attr[order]
    core_of = d_s // nloc
    gbkt = d_s // B
    bkt_loc = gbkt - core_of * nbkt
    counts = np.bincount(core_of * nbkt + bkt_loc, minlength=nbkt * n_cores)
    cap = int(-(-counts.max() // P))
    tt = nbkt * cap

    starts = np.zeros(nbkt * n_cores + 1, np.int64)
    np.cumsum(counts, out=starts[1:])
    rank = np.arange(len(d_s), dtype=np.int64) - starts[core_of * nbkt + bkt_loc]
    slot = bkt_loc * (cap * P) + rank
    lane = slot % P
    tile = slot // P

    blk_t = np.zeros((n_cores, P, tt), np.int16)       # pad -> block 0
    sm4_t = np.full((n_cores, P, tt), -1.0, np.float32)  # pad -> -1
    dl_t = np.full((n_cores, P, tt), -1.0, np.float32)
    at_t = np.zeros((n_cores, P, tt, 4), np.float32)
    blk_t[core_of, lane, tile] = (s_s >> 2).astype(np.int16)
    sm4_t[core_of, lane, tile] = (s_s & 3).astype(np.float32)
    dl_t[core_of, lane, tile] = (d_s % B).astype(np.float32)
    at_t[core_of, lane, tile] = a_s

    # wrap block ids into the dma_gather index layout: flat index i=t*128+p
    # (within a CT-tile chunk) lives at [i%16, i//16], replicated over the 8
    # groups of 16 partitions.  [C,P,tt] -> [C,128,tt*8] int16.
    CT = CG * cap
    nch = nbkt // CG
    b5 = blk_t.reshape(n_cores, 8, 16, nch, CT)        # p = q*16 + r
    b5 = b5.transpose(0, 3, 2, 4, 1)                   # [C, nch, r, t, q]
    b5 = b5.reshape(n_cores, nch, 16, CT * 8)
    idx16 = np.broadcast_to(b5[:, :, None, :, :],
                            (n_cores, nch, 8, 16, CT * 8))
    idx16 = idx16.transpose(0, 2, 3, 1, 4).reshape(n_cores, 128, tt * 8)
    idx16 = np.ascontiguousarray(idx16)

    cfg = dict(nloc=nloc, nbkt=nbkt, cap=cap, tt=tt, n_pad=n_pad,
               n_cores=n_cores)
    return cfg, idx16, sm4_t, dl_t, at_t


def _householder(a):
    """Orthogonal symmetric involution R with R[:, -1] = a/|a|."""
    h = a.shape[0]
    v = a / np.linalg.norm(a)
    e = np.zeros(h)
    e[h - 1] = 1.0
    u = e - v
    n = np.linalg.norm(u)
    if n < 1e-7:
        return np.eye(h)
    u = u / n
    return np.eye(h) - 2.0 * np.outer(u, u)


# ------------------------------------------------------------- device program
def _build_program(cfg):
    import concourse.bass as bass
    import concourse.mybir as mybir
    import concourse.tile as tile
    from concourse import bacc
    from concourse.masks import make_identity
    from contextlib import ExitStack

    f32 = mybir.dt.float32
    bf16 = mybir.dt.bfloat16
    i16 = mybir.dt.int16
    AT = mybir.AluOpType
    AF = mybir.ActivationFunctionType
    AX = mybir.AxisListType

    nloc, nbkt, cap, tt = cfg["nloc"], cfg["nbkt"], cfg["cap"], cfg["tt"]
    n_pad, n_cores = cfg["n_pad"], cfg["n_cores"]
    anorm = cfg["anorm"]                 # (|c1_att_src|, |c2_att_src|)
    nchunk = nbkt // CG
    CT = CG * cap                    # edge tiles per chunk
    CN = CG * B                      # nodes per chunk (128)
    NCH = nloc // P                  # node-major chunks
    NC_A = nloc // NCA               # f-major chunks
    NB = n_pad // 4                  # 4-node gather blocks
    groups = [list(range(n_cores))]

    nc = bacc.Bacc("TRN2", target_bir_lowering=False, debug=False,
                   num_devices=n_cores)

    # ---------------- external inputs
    xT = nc.dram_tensor("xT", [5, nloc], f32, kind="ExternalInput")
    idx_d = nc.dram_tensor("idx", [P, tt * 8], i16, kind="ExternalInput")
    sm4_d = nc.dram_tensor("sm4", [P, tt], bf16, kind="ExternalInput")
    dl_d = nc.dram_tensor("dl", [P, tt], bf16, kind="ExternalInput")
    at_d = nc.dram_tensor("attr", [P, tt, 4], f32, kind="ExternalInput")
    wnames = [("enc_W", [5, 32]), ("enc_b", [32, 1]),
              ("c1_WY", [32, 32]), ("c1_adY", [32, 1]), ("c1_R", [32, 32]),
              ("c1_adYr", [1, 32]), ("c2_adYr", [1, 32]),
              ("c1_weatte", [1, 4]), ("c1_b", [32, 1]),
              ("c2_WY", [32, 32]), ("c2_adY", [32, 1]),
              ("c2_weatte", [1, 4]),
              ("dec_WT", [1, 128]), ("dec_b", [1, 4])]
    wh = {n: nc.dram_tensor(n, s, f32, kind="ExternalInput")
          for n, s in wnames}
    out_d = nc.dram_tensor("out", [nloc, 4], f32, kind="ExternalOutput")

    # ---------------- internal DRAM
    tab_own = [nc.dram_tensor(f"tab_own{i}", [nloc, 32], bf16)
               for i in range(2)]
    tab_full = [nc.dram_tensor(f"tab_full{i}", [NB, 128], bf16,
                               addr_space="Shared") for i in range(2)]
    num_d = [nc.dram_tensor(f"num{i}", [nloc // P, P, 36], f32)
             for i in range(2)]
    ea2_d = nc.dram_tensor("ea2buf", [P, tt], f32)

    with tile.TileContext(nc) as tc, ExitStack() as ctx:
        const = ctx.enter_context(tc.tile_pool(name="const", bufs=1))
        keep = ctx.enter_context(tc.tile_pool(name="keep", bufs=1))
        nodef = ctx.enter_context(tc.tile_pool(name="nodef", bufs=2))
        psn = ctx.enter_context(tc.tile_pool(name="psn", bufs=1,
                                             space="PSUM"))
        pse = ctx.enter_context(tc.tile_pool(name="pse", bufs=2,
                                             space="PSUM"))
        edge = ctx.enter_context(tc.tile_pool(name="edge", bufs=2))
        nph = ctx.enter_context(tc.tile_pool(name="nph", bufs=1))

        # ---------- constants
        iota_i = const.tile([P, B], mybir.dt.int32)
        nc.gpsimd.iota(iota_i[:], pattern=[[1, B]], base=0,
                       channel_multiplier=0)
        iota16 = const.tile([P, B], bf16)
        nc.vector.tensor_copy(iota16[:], iota_i[:])
        iota4 = const.tile([P, 4], bf16)
        nc.vector.tensor_copy(iota4[:], iota_i[:, 0:4])
        ones_row = const.tile([1, P], f32)
        nc.vector.memset(ones_row[:], 1.0)
        ones16 = const.tile([1, P], bf16)
        nc.vector.memset(ones16[:], 1.0)
        ident = const.tile([P, P], bf16)
        make_identity(nc, ident[:])

        sbw = {}
        for n, s in wnames:
            t = const.tile(s, f32, tag=f"w_{n}")
            nc.sync.dma_start(t[:], wh[n][:])
            sbw[n] = t

        def bcast_row(row_ap, n, out_dt, pool, tag):
            """[1, n] f32 row -> [P, n] tile via PE outer product."""
            ps = psn.tile([P, 512], f32, tag="psb")
            ones = ones16 if row_ap.dtype == bf16 else ones_row
            nc.tensor.matmul(ps[:, 0:n], lhsT=ones[:], rhs=row_ap,
                             start=True, stop=True)
            out = pool.tile([P, n], out_dt, tag=tag)
            nc.scalar.copy(out[:], ps[:, 0:n])
            return out

        we_bc = [bcast_row(sbw["c1_weatte"][:], 4, f32, const, "webc0"),
                 bcast_row(sbw["c2_weatte"][:], 4, f32, const, "webc1")]
        wdall = bcast_row(sbw["dec_WT"][:], 128, f32, const, "wdbc")
        wd_bc = [wdall[:, 32 * k:32 * (k + 1)] for k in range(4)]
        bd_bc = bcast_row(sbw["dec_b"][:], 4, f32, const, "bdbc")

        sbw16 = {}
        for n in ("c1_WY", "c1_adY", "c1_R", "c2_WY", "c2_adY"):
            t16 = const.tile(sbw[n].shape, bf16, tag=f"w16_{n}")
            nc.vector.tensor_copy(t16[:], sbw[n][:])
            sbw16[n] = t16

        # ---------- stage A: encoder (f-major)
        h0T = keep.tile([32, nloc], bf16, tag="hT")
        for c in range(NC_A):
            sl = slice(c * NCA, (c + 1) * NCA)
            xc = nodef.tile([5, NCA], f32, tag="xc")
            nc.sync.dma_start(xc[:], xT[:, sl])
            ps = psn.tile([P, NCA], f32, tag="psn")
            nc.tensor.matmul(ps[0:32, :NCA], lhsT=sbw["enc_W"][:],
                             rhs=xc[:], start=True, stop=True)
            nc.scalar.activation(h0T[:, sl], ps[0:32, :NCA], AF.Relu,
                                 bias=sbw["enc_b"][:], scale=1.0)

        def make_table(hT, wyk, adk, adrk, conv):
            """f-major hidden [32, nloc] -> node-major y1 table + keeps."""
            y1T = keep.tile([32, nloc], bf16, tag="y1T")
            for c in range(NC_A):
                sl = slice(c * NCA, (c + 1) * NCA)
                ps = psn.tile([P, NCA], f32, tag="psn")
                nc.tensor.matmul(ps[0:32, :NCA], lhsT=sbw16[wyk][:],
                                 rhs=hT[:, sl], start=True, stop=True)
                nc.scalar.copy(y1T[:, sl], ps[0:32, :NCA])
            stag = keep.tile([P, NCH, 34], bf16, tag="stag")
            for c in range(NCH):
                ps = psn.tile([P, P], bf16, tag="psnT")
                nc.tensor.transpose(out=ps[:, 0:32],
                                    in_=y1T[:, c * P:(c + 1) * P],
                                    identity=ident[0:32, 0:32])
                nc.scalar.copy(stag[:, c, 0:32], ps[:, 0:32])
            # a_src = |att_src| * y1[:,31]  (node-major, col 32)
            nc.vector.tensor_scalar_mul(stag[:, :, 32], stag[:, :, 31],
                                        float(anorm[conv]))
            # a_dst = y1 @ adY  (node-major, col 33)
            adr_bc = bcast_row(sbw[adrk][:], 32, bf16, keep, "adrbc")
            adt = nph.tile([P, NCH, 32], bf16, tag="adt")
            nc.vector.tensor_tensor(
                out=adt[:], in0=stag[:, :, 0:32],
                in1=adr_bc[:, None, :].to_broadcast([P, NCH, 32]),
                op=AT.mult)
            adf = nph.tile([P, NCH], f32, tag="adf")
            nc.vector.tensor_reduce(out=adf[:], in_=adt[:],
                                    axis=AX.X, op=AT.add)
            nc.vector.tensor_copy(stag[:, :, 33], adf[:])
            # adY broadcast [32, CN] - lhsT for per-chunk a_dst row matmuls
            adY_bc = keep.tile([32, CN], bf16, tag="adYbc")
            nc.vector.tensor_copy(adY_bc[:],
                                  sbw16[adk][:].to_broadcast([32, CN]))
            return y1T, adY_bc, stag

        def publish_table(stag, conv):
            own_view = tab_own[conv][:].rearrange("(c p) r -> p c r", p=P)
            nc.sync.dma_start(own_view, stag[:, :, 0:32])
            nc.gpsimd.collective_compute(
                "AllGather", mybir.AluOpType.bypass,
                replica_groups=groups,
                ins=[tab_own[conv][:]],
                outs=[tab_full[conv][:]],
            )

        y1T1, adY_bc1, stag1 = make_table(h0T, "c1_WY", "c1_adY",
                                          "c1_adYr", 0)
        publish_table(stag1, 0)

        # ---------- edge pipeline
        def edge_pass(conv, y1T, adY_bc):
            NRW = 132 if conv == 0 else 129      # 4*32 quarters + extras
            for ch in range(nchunk):
                tsl = slice(ch * CT, (ch + 1) * CT)
                isl = slice(ch * CT * 8, (ch + 1) * CT * 8)
                idx_s = edge.tile([P, CT * 8], i16, tag="idx")
                nc.sync.dma_start(idx_s[:], idx_d[:, isl])
                dls = edge.tile([P, CT], bf16, tag="dls")
                nc.sync.dma_start(dls[:], dl_d[:, tsl])
                sm4 = edge.tile([P, CT], bf16, tag="sm4")
                nc.sync.dma_start(sm4[:], sm4_d[:, tsl])
                vp4 = edge.tile([P, CT, 128], bf16, tag="v4", bufs=3)
                nc.gpsimd.dma_gather(
                    out_ap=vp4[:], in_ap=tab_full[conv][:],
                    idxs_ap=idx_s[:], num_idxs=CT * P,
                    num_idxs_reg=CT * P, elem_size=128,
                    single_packet=False)
                v4q = vp4[:].rearrange("p t (q k) -> p t q k", q=4)
                oh = edge.tile([P, CT, B], bf16, tag="oh")
                nc.vector.tensor_tensor(
                    out=oh[:], in0=dls[:].to_broadcast([P, CT, B]),
                    in1=iota16[:, None, :].to_broadcast([P, CT, B]),
                    op=AT.is_equal)
                m4 = edge.tile([P, CT, 4], bf16, tag="m4")
                nc.vector.tensor_tensor(
                    out=m4[:], in0=sm4[:].to_broadcast([P, CT, 4]),
                    in1=iota4[:, None, :].to_broadcast([P, CT, 4]),
                    op=AT.is_equal)
                a32ps = psn.tile([P, 512], f32, tag="psb")
                nc.tensor.matmul(a32ps[:, 0:CN], lhsT=adY_bc[:],
                                 rhs=y1T[:, ch * CN:(ch + 1) * CN],
                                 start=True, stop=True)
                a32 = edge.tile([P, CN], bf16, tag="a32")
                nc.scalar.copy(a32[:], a32ps[:, 0:CN])
                dprod = edge.tile([P, CT, B], bf16, tag="dprod")
                a32v = a32[:].rearrange("p (g b) -> p g b", b=B)
                nc.vector.tensor_tensor(
                    out=dprod[:].rearrange("p (g c) b -> p g c b", c=cap),
                    in0=oh[:].rearrange("p (g c) b -> p g c b", c=cap),
                    in1=a32v[:, :, None, :].to_broadcast([P, CG, cap, B]),
                    op=AT.mult)
                dexp = edge.tile([P, CT], f32, tag="dexp")
                nc.vector.tensor_reduce(out=dexp[:], in_=dprod[:],
                                        axis=AX.X, op=AT.add)
                ea = edge.tile([P, CT], f32, tag="ea")
                if conv == 0:
                    at_s = edge.tile([P, CT, 4], f32, tag="at")
                    nc.sync.dma_start(at_s[:], at_d[:, tsl, :])
                    ea4 = edge.tile([P, CT, 4], f32, tag="ea4")
                    nc.vector.tensor_tensor(
                        out=ea4[:], in0=at_s[:],
                        in1=we_bc[0][:, None, :].to_broadcast([P, CT, 4]),
                        op=AT.mult)
                    nc.vector.tensor_reduce(out=ea[:], in_=ea4[:],
                                            axis=AX.X, op=AT.add)
                    ea2 = edge.tile([P, CT], f32, tag="ea2")
                    ea4b = edge.tile([P, CT, 4], f32, tag="ea4")
                    nc.vector.tensor_tensor(
                        out=ea4b[:], in0=at_s[:],
                        in1=we_bc[1][:, None, :].to_broadcast([P, CT, 4]),
                        op=AT.mult)
                    nc.vector.tensor_reduce(out=ea2[:], in_=ea4b[:],
                                            axis=AX.X, op=AT.add)
                    nc.sync.dma_start(ea2_d[:, tsl], ea2[:])
                else:
                    nc.sync.dma_start(ea[:], ea2_d[:, tsl])
                # a_src = |a| * y1[src][31], quarter-selected via m4
                sfq = edge.tile([P, CT, 4], f32, tag="sfq")
                nc.vector.tensor_tensor(out=sfq[:], in0=v4q[:, :, :, 31],
                                        in1=m4[:], op=AT.mult)
                z = edge.tile([P, CT], f32, tag="z")
                nc.vector.tensor_reduce(out=z[:], in_=sfq[:], axis=AX.X,
                                        op=AT.add)
                nc.vector.tensor_scalar_mul(z[:], z[:], float(anorm[conv]))
                nc.vector.tensor_tensor(out=z[:], in0=z[:], in1=ea[:],
                                        op=AT.add)
                nc.vector.tensor_tensor(out=z[:], in0=z[:], in1=dexp[:],
                                        op=AT.add)
                nc.vector.tensor_scalar_min(z[:], z[:], ZCLAMP)
                zn = edge.tile([P, CT], f32, tag="zn")
                nc.vector.tensor_scalar_mul(zn[:], z[:], NEG)
                nc.vector.tensor_tensor(out=z[:], in0=z[:], in1=zn[:],
                                        op=AT.max)
                ex = edge.tile([P, CT], bf16, tag="ex")
                nc.scalar.activation(ex[:], z[:], AF.Exp)
                exq = edge.tile([P, CT, 4], bf16, tag="exq")
                nc.vector.tensor_tensor(
                    out=exq[:], in0=m4[:],
                    in1=ex[:, :, None].to_broadcast([P, CT, 4]),
                    op=AT.mult)
                rhs = edge.tile([P, CT, 132], bf16, tag="rhs")
                nc.vector.tensor_tensor(
                    out=rhs[:, :, 0:128].rearrange("p t (q k) -> p t q k",
                                                   q=4),
                    in0=v4q[:],
                    in1=exq[:, :, :, None].to_broadcast([P, CT, 4, 32]),
                    op=AT.mult)
                nc.vector.tensor_copy(rhs[:, :, 128], ex[:])
                if conv == 0:
                    nc.vector.tensor_copy(rhs[:, :, 129], ea[:])
                    nc.vector.tensor_copy(rhs[:, :, 130], ea2[:])
                    nc.vector.memset(rhs[:, :, 131], 1.0)
                psv = [pse.tile([B, 2 * NRW], f32, tag="pseA", name="pseA"),
                       pse.tile([B, 2 * NRW], f32, tag="pseB", name="pseB")]
                for g in range(CG):
                    ps = psv[g // 2]
                    off = (g % 2) * NRW
                    for i in range(cap):
                        t = g * cap + i
                        nc.tensor.matmul(
                            ps[:, off:off + NRW],
                            lhsT=oh[:, t, :], rhs=rhs[:, t, 0:NRW],
                            start=(i == 0), stop=(i == cap - 1))
                st = edge.tile([B, CG, 36], f32, tag="st")
                NR = 36 if conv == 0 else 33
                for g in range(CG):
                    ps = psv[g // 2]
                    off = (g % 2) * NRW
                    nc.vector.tensor_reduce(
                        out=st[:, g, 0:32],
                        in_=ps[:, off:off + 128].rearrange(
                            "b (q k) -> b k q", q=4),
                        axis=AX.X, op=AT.add)
                    nc.vector.tensor_copy(st[:, g, 32:NR],
                                          ps[:, off + 128:off + NRW])
                nc.sync.dma_start(
                    num_d[conv][ch].rearrange("(g b) r -> b g r", b=B)
                    [:, :, 0:NR],
                    st[:, :, 0:NR])

        edge_pass(0, y1T1, adY_bc1)

        # ---------- node phase
        keepn = keep.tile([P, NCH, 2], f32, tag="keepn")

        def node_finish(conv, stag):
            """-> hy [P, NCH, 32] f32: rotated-basis output, no bias/relu."""
            NR = 36 if conv == 0 else 33
            num = nph.tile([P, NCH, NR], f32, tag="num")
            nc.sync.dma_start(
                num[:],
                num_d[conv][:].rearrange("c p r -> p c r")[:, :, 0:NR])
            if conv == 0:
                easum_ap = num[:, :, 33]
                deg_ap = num[:, :, 35]
                nc.vector.tensor_copy(keepn[:, :, 0], num[:, :, 34])
                nc.vector.tensor_copy(keepn[:, :, 1], num[:, :, 35])
            else:
                easum_ap = keepn[:, :, 0]
                deg_ap = keepn[:, :, 1]
            dg = nph.tile([P, NCH], f32, tag="dg")
            nc.vector.tensor_scalar_max(dg[:], deg_ap, 1.0)
            nc.vector.reciprocal(dg[:], dg[:])
            zl = nph.tile([P, NCH], f32, tag="zl2")
            nc.vector.tensor_tensor(out=zl[:], in0=easum_ap, in1=dg[:],
                                    op=AT.mult)
            asf = nph.tile([P, NCH], f32, tag="asf")
            nc.vector.tensor_copy(asf[:], stag[:, :, 32])
            nc.vector.tensor_tensor(out=zl[:], in0=zl[:], in1=asf[:],
                                    op=AT.add)
            nc.vector.tensor_copy(asf[:], stag[:, :, 33])
            nc.vector.tensor_tensor(out=zl[:], in0=zl[:], in1=asf[:],
                                    op=AT.add)
            zln = nph.tile([P, NCH], f32, tag="zln")
            nc.vector.tensor_scalar_mul(zln[:], zl[:], NEG)
            nc.vector.tensor_tensor(out=zl[:], in0=zl[:], in1=zln[:],
                                    op=AT.max)
            exl = nph.tile([P, NCH], f32, tag="exl")
            nc.scalar.activation(exl[:], zl[:], AF.Exp)
            den = nph.tile([P, NCH], f32, tag="den")
            nc.vector.tensor_tensor(out=den[:], in0=num[:, :, 32],
                                    in1=exl[:], op=AT.add)
            nc.vector.reciprocal(den[:], den[:])
            hy = nph.tile([P, NCH, 32], f32, tag="h")
            exl16 = nph.tile([P, NCH], bf16, tag="exl16")
            nc.vector.tensor_copy(exl16[:], exl[:])
            nc.vector.tensor_tensor(
                out=hy[:], in0=stag[:, :, 0:32],
                in1=exl16[:, :, None].to_broadcast([P, NCH, 32]),
                op=AT.mult)
            nc.vector.tensor_tensor(out=hy[:], in0=hy[:],
                                    in1=num[:, :, 0:32], op=AT.add)
            nc.vector.tensor_tensor(
                out=hy[:], in0=hy[:],
                in1=den[:, :, None].to_broadcast([P, NCH, 32]), op=AT.mult)
            return hy

        hy1 = node_finish(0, stag1)

        # node-major -> f-major chunk-wise; un-rotate h1 = relu(hy1@R1 + b1)
        h1b = nph.tile([P, NCH, 32], bf16, tag="h1b")
        nc.vector.tensor_copy(h1b[:], hy1[:])
        h1T = keep.tile([32, nloc], bf16, tag="hT")
        for c in range(NCH):
            ps = psn.tile([P, P], bf16, tag="psnT")
            nc.tensor.transpose(out=ps[0:32, 0:P], in_=h1b[:, c, :],
                                identity=ident[:])
            hst = nodef.tile([32, P], bf16, tag="hst")
            nc.scalar.copy(hst[:], ps[0:32, 0:P])
            ps2 = psn.tile([P, P], f32, tag="psnR")
            nc.tensor.matmul(ps2[0:32, 0:P], lhsT=sbw16["c1_R"][:],
                             rhs=hst[:], start=True, stop=True)
            nc.scalar.activation(h1T[:, c * P:(c + 1) * P], ps2[0:32, 0:P],
                                 AF.Relu, bias=sbw["c1_b"][:], scale=1.0)

        y1T2, adY_bc2, stag2 = make_table(h1T, "c2_WY", "c2_adY",
                                          "c2_adYr", 1)
        publish_table(stag2, 1)
        edge_pass(1, y1T2, adY_bc2)
        hy2 = node_finish(1, stag2)

        # ---------- decoder + log_softmax (node-major; dec weights carry
        # the conv2 un-rotation and bias fold)
        lg = nph.tile([P, NCH, 4], f32, tag="lg")
        tmp = nph.tile([P, NCH, 32], f32, tag="dtmp")
        for k in range(4):
            nc.vector.tensor_tensor(
                out=tmp[:], in0=hy2[:],
                in1=wd_bc[k][:, None, :].to_broadcast([P, NCH, 32]),
                op=AT.mult)
            nc.vector.tensor_reduce(out=lg[:, :, k], in_=tmp[:], axis=AX.X,
                                    op=AT.add)
        nc.vector.tensor_tensor(
            out=lg[:], in0=lg[:],
            in1=bd_bc[:, None, 0:4].to_broadcast([P, NCH, 4]), op=AT.add)
        mx = nph.tile([P, NCH], f32, tag="mx")
        nc.vector.tensor_reduce(out=mx[:], in_=lg[:], axis=AX.X, op=AT.max)
        nc.vector.tensor_tensor(
            out=lg[:], in0=lg[:],
            in1=mx[:, :, None].to_broadcast([P, NCH, 4]), op=AT.subtract)
        el = nph.tile([P, NCH, 4], f32, tag="el")
        nc.scalar.activation(el[:], lg[:], AF.Exp)
        se = nph.tile([P, NCH], f32, tag="se")
        nc.vector.tensor_reduce(out=se[:], in_=el[:], axis=AX.X, op=AT.add)
        ls = nph.tile([P, NCH], f32, tag="ls")
        nc.scalar.activation(ls[:], se[:], AF.Ln)
        nc.vector.tensor_tensor(
            out=lg[:], in0=lg[:],
            in1=ls[:, :, None].to_broadcast([P, NCH, 4]), op=AT.subtract)
        nc.sync.dma_start(
            out_d[:].rearrange("(c p) r -> p c r", p=P), lg[:])

    nc.compile()
    return nc


_PROGRAM_CACHE = {}


def _get_program(cfg):
    key = (cfg["nloc"], cfg["cap"], cfg["anorm"])
    if key not in _PROGRAM_CACHE:
        _PROGRAM_CACHE[key] = _build_program(cfg)
    return _PROGRAM_CACHE[key]


def _make_in_maps(inputs, cfg, idx16, sm4_t, dl_t, at_t):
    import ml_dtypes
    f32 = np.float32
    x = np.asarray(inputs["x"], f32)
    nloc, n_pad, n_cores = cfg["nloc"], cfg["n_pad"], cfg["n_cores"]
    xp = np.zeros((n_pad, 5), f32)
    xp[:x.shape[0]] = x

    a1 = np.asarray(inputs["c1_att_src"], np.float64)
    a2 = np.asarray(inputs["c2_att_src"], np.float64)
    R1 = _householder(a1)
    R2 = _householder(a2)
    W1 = np.asarray(inputs["c1_W"], np.float64)
    W2 = np.asarray(inputs["c2_W"], np.float64)
    decW = np.asarray(inputs["dec_W"], np.float64)
    wdY = R2 @ decW                                     # [32, 4]
    bY = (np.asarray(inputs["c2_b"], np.float64) @ decW
          + np.asarray(inputs["dec_b"], np.float64))    # [4]

    com = {
        "enc_W": np.asarray(inputs["enc_W"], f32),
        "enc_b": np.asarray(inputs["enc_b"], f32).reshape(32, 1),
        "c1_WY": (W1 @ R1).astype(f32),
        "c1_adY": (R1 @ np.asarray(inputs["c1_att_dst"], np.float64)
                   ).astype(f32).reshape(32, 1),
        "c1_adYr": (R1 @ np.asarray(inputs["c1_att_dst"], np.float64)
                    ).astype(f32).reshape(1, 32),
        "c2_adYr": (R2 @ np.asarray(inputs["c2_att_dst"], np.float64)
                    ).astype(f32).reshape(1, 32),
        "c1_R": R1.astype(f32),
        "c1_weatte": (np.asarray(inputs["c1_We"], f32)
                      @ np.asarray(inputs["c1_att_e"], f32)).reshape(1, 4),
        "c1_b": np.asarray(inputs["c1_b"], f32).reshape(32, 1),
        "c2_WY": (W2 @ R2).astype(f32),
        "c2_adY": (R2 @ np.asarray(inputs["c2_att_dst"], np.float64)
                   ).astype(f32).reshape(32, 1),
        "c2_weatte": (np.asarray(inputs["c2_We"], f32)
                      @ np.asarray(inputs["c2_att_e"], f32)).reshape(1, 4),
        "dec_WT": wdY.T.astype(f32).copy().reshape(1, 128),
        "dec_b": bY.astype(f32).reshape(1, 4),
    }
    in_maps = []
    for c in range(n_cores):
        m = dict(com)
        m["xT"] = xp[c * nloc:(c + 1) * nloc].T.copy()
        m["idx"] = idx16[c]
        m["sm4"] = sm4_t[c].astype(ml_dtypes.bfloat16)
        m["dl"] = dl_t[c].astype(ml_dtypes.bfloat16)
        m["attr"] = at_t[c]
        in_maps.append(m)
    return in_maps


# ------------------------------------------------------------------ entrypoint
def kernel(**inputs):
    ei = np.asarray(inputs["edge_index"])
    attr = np.asarray(inputs["edge_attr"], np.float32)
    n_trucks = int(inputs["num_trucks"])
    n_nodes = np.asarray(inputs["x"]).shape[0]
    n_cores = 8

    src = ei[0].astype(np.int32)
    dst = ei[1].astype(np.int32)
    cfg, idx16, sm4_t, dl_t, at_t = _build_layout(src, dst, attr, n_nodes,
                                                  n_cores)
    cfg["anorm"] = (float(np.linalg.norm(inputs["c1_att_src"])),
                    float(np.linalg.norm(inputs["c2_att_src"])))
    in_maps = _make_in_maps(inputs, cfg, idx16, sm4_t, dl_t, at_t)

    nc = _get_program(cfg)
    from concourse.bass_utils import run_bass_kernel_spmd
    res = run_bass_kernel_spmd(nc, in_maps, core_ids=list(range(n_cores)),
                               trace=False)
    outs = [res.results[c]["out"] for c in range(n_cores)]
    full = np.concatenate(outs, axis=0)[:n_trucks]
    return np.asarray(full, np.float32)
